# revision 1
# baseline (speedup 1.0000x reference)
"""Swin BasicLayer (depth=2 windowed attention) Trainium2 kernel.

Sharding: data-parallel over batch B=8 across 8 NeuronCores; weights
replicated. Each core runs both depths (regular + shifted windows) over
its [12544, 384] image in [C, token] layout.
"""
import numpy as np
import ml_dtypes

import concourse.bass as bass
import concourse.tile as tile
from concourse import bacc, mybir
from concourse.bass_utils import run_bass_kernel_spmd

f32 = mybir.dt.float32
f32r = mybir.dt.float32r
bf16 = mybir.dt.bfloat16
AF = mybir.ActivationFunctionType
ALU = mybir.AluOpType

B, H, W, C = 8, 112, 112, 384
NH, HD, WS = 12, 32, 7
N = WS * WS          # 49
L = H * W            # 12544
NBAND = H // WS      # 16
BAND = WS * W        # 784
HALF = BAND // 2     # 392
NWB = W // WS        # 16 windows per band
NG = NWB // 2        # 8 window-pair groups
DEPTH = 2


def _round_f32r(x):
    v = np.ascontiguousarray(x, np.float32).view(np.uint32)
    v = ((v.astype(np.uint64) + 0x800) & 0xFFFFF000).astype(np.uint32)
    return v.view(np.float32)


def _rel_pos_index():
    coords = np.stack(np.meshgrid(np.arange(WS), np.arange(WS), indexing='ij')).reshape(2, -1)
    rel = (coords[:, :, None] - coords[:, None, :]).transpose(1, 2, 0).copy()
    rel[..., 0] += WS - 1
    rel[..., 1] += WS - 1
    rel[..., 0] *= 2 * WS - 1
    return rel.sum(-1)


def _masks():
    """The 4 distinct [N, N] shifted-window masks by type 2*(i==15)+(j==15)."""
    ws, shift = WS, WS // 2
    img = np.zeros((H, W), dtype=np.float32)
    slices = (slice(0, -ws), slice(-ws, -shift), slice(-shift, None))
    cnt = 0
    for hs in slices:
        for wsl in slices:
            img[hs, wsl] = cnt
            cnt += 1
    mw = img.reshape(H // ws, ws, W // ws, ws).transpose(0, 2, 1, 3).reshape(-1, ws * ws)
    diff = mw[:, None, :] - mw[:, :, None]
    mask = np.where(diff != 0, -100.0, 0.0).astype(np.float32)  # [nW, N, N]
    nwr = H // ws
    m = {}
    for ti, widx in ((0, 0), (1, nwr - 1), (2, (nwr - 1) * nwr), (3, nwr * nwr - 1)):
        m[ti] = mask[widx]
    return m


def _band_ranges(i, shift):
    r0 = (WS * i + shift) % H
    n0 = min(WS, H - r0)
    rows = [(0, r0, n0)]
    if n0 < WS:
        rows.append((n0, 0, WS - n0))
    if shift == 0:
        cols = [(0, 0, W)]
    else:
        cols = [(0, shift, W - shift), (W - shift, 0, shift)]
    return rows, cols


def _build(nbands=NBAND, ndepth=DEPTH, simsafe=False, attn_dt=bf16):
    nc = bacc.Bacc("TRN2", target_bir_lowering=False, debug=False, num_devices=8)

    xin = nc.dram_tensor("xin", [C, L], f32r, kind="ExternalInput")
    xmid = nc.dram_tensor("xmid", [C, L], f32r)
    xout = nc.dram_tensor("xout", [C, L], f32, kind="ExternalOutput")
    dwqk = nc.dram_tensor("wqk", [DEPTH, C, 2 * C], f32r, kind="ExternalInput")
    dwv = nc.dram_tensor("wv", [DEPTH, C, C], f32r, kind="ExternalInput")
    dwp = nc.dram_tensor("wp", [DEPTH, C, C], f32r, kind="ExternalInput")
    dbqk = nc.dram_tensor("bqk", [DEPTH, 2 * C], f32, kind="ExternalInput")
    dbp = nc.dram_tensor("bp", [DEPTH, C], f32, kind="ExternalInput")
    de0 = nc.dram_tensor("e0", [128, 12 * N], f32, kind="ExternalInput")
    de1 = nc.dram_tensor("e1", [128, 4 * 12 * N], f32, kind="ExternalInput")
    didn = nc.dram_tensor("idn", [128, N], attn_dt, kind="ExternalInput")

    def evac_copy(use_dve, out_ap, in_ap):
        if use_dve:
            nc.vector.tensor_copy(out_ap, in_ap)
        else:
            nc.scalar.activation(out_ap, in_ap, AF.Identity, bias=0.0)

    def evac_bias(use_dve, out_ap, in_ap, bias_ap):
        if use_dve:
            nc.vector.tensor_scalar_add(out_ap, in_ap, bias_ap)
        else:
            nc.scalar.activation(out_ap, in_ap, AF.Identity, bias=bias_ap)

    with tile.TileContext(nc) as tc:
        cpool = tc.alloc_tile_pool(name="const", bufs=1)
        p_xr = tc.alloc_tile_pool(name="xr", bufs=4)
        p_xw = tc.alloc_tile_pool(name="xw", bufs=6)
        p_qkb = tc.alloc_tile_pool(name="qkb", bufs=8)
        p_outT = tc.alloc_tile_pool(name="outT", bufs=6)
        p_yr = tc.alloc_tile_pool(name="yr", bufs=4)
        p_pt = tc.alloc_tile_pool(name="pt", bufs=3)
        p_vt = tc.alloc_tile_pool(name="vt", bufs=8)
        p_on = tc.alloc_tile_pool(name="onat", bufs=3)
        p_rt = tc.alloc_tile_pool(name="rt", bufs=3)
        ps_proj = tc.alloc_tile_pool(name="psproj", bufs=1, space="PSUM")
        ps_v = tc.alloc_tile_pool(name="psv", bufs=2, space="PSUM")
        ps_s = tc.alloc_tile_pool(name="pss", bufs=1, space="PSUM")
        ps_av = tc.alloc_tile_pool(name="psav", bufs=1, space="PSUM")

        # constants
        wqk = [[cpool.tile([128, 2 * C], f32r, tag=f"wqk{d}{cc}", name=f"wqk{d}{cc}") for cc in range(3)]
               for d in range(DEPTH)]
        wv = [[cpool.tile([128, C], f32r, tag=f"wv{d}{cc}", name=f"wv{d}{cc}") for cc in range(3)]
              for d in range(DEPTH)]
        wp = [[cpool.tile([128, C], f32r, tag=f"wp{d}{cc}", name=f"wp{d}{cc}") for cc in range(3)]
              for d in range(DEPTH)]
        bqk = [cpool.tile([128, 6], f32, tag=f"bqk{d}", name=f"bqk{d}") for d in range(DEPTH)]
        bp = [cpool.tile([128, 3], f32, tag=f"bp{d}", name=f"bp{d}") for d in range(DEPTH)]
        e0 = cpool.tile([128, 12 * N], f32, tag="e0", name="e0")
        e1 = cpool.tile([128, 4 * 12 * N], f32, tag="e1", name="e1")
        idn = cpool.tile([128, N], attn_dt, tag="idn", name="idn")
        for d in range(DEPTH):
            for cc in range(3):
                nc.sync.dma_start(wqk[d][cc][:], dwqk[d, cc * 128:(cc + 1) * 128, :])
                nc.sync.dma_start(wv[d][cc][:], dwv[d, cc * 128:(cc + 1) * 128, :])
                nc.sync.dma_start(wp[d][cc][:], dwp[d, cc * 128:(cc + 1) * 128, :])
            nc.sync.dma_start(bqk[d][:], dbqk[d].rearrange("(o p) -> p o", p=128))
            nc.sync.dma_start(bp[d][:], dbp[d].rearrange("(o p) -> p o", p=128))
        nc.sync.dma_start(e0[:], de0[:])
        nc.sync.dma_start(e1[:], de1[:])
        nc.sync.dma_start(idn[:], didn[:])

        for d in range(ndepth):
            shift = 0 if d == 0 else WS // 2
            xsrc = xin if d == 0 else xmid
            if d == ndepth - 1:
                ydst, ydt = xout, f32
            else:
                ydst, ydt = xmid, f32r
            for bi in range(nbands):
                rows, cols = _band_ranges(bi, shift)
                # ---- load band (raster) + window-major relayout
                xr = [p_xr.tile([128, BAND], f32r, tag="xr", name="xr") for _ in range(3)]
                xw = [p_xw.tile([128, BAND], f32r, tag="xw", name="xw") for _ in range(3)]
                for cc in range(3):
                    xr3 = xr[cc][:].rearrange("p (r c) -> p r c", r=WS)
                    src3 = xsrc[cc * 128:(cc + 1) * 128, :].rearrange(
                        "p (r c) -> p r c", r=H)
                    for (dr, sr, nr) in rows:
                        for (dc, sc, ncl) in cols:
                            nc.sync.dma_start(xr3[:, dr:dr + nr, dc:dc + ncl],
                                              src3[:, sr:sr + nr, sc:sc + ncl])
                    # raster (r, 7w+j) -> window-major (49w + 7r + j)
                    in_ap = xr[cc][:].rearrange("p (r w j) -> p w r j", r=WS, w=NWB, j=WS)
                    out_ap = xw[cc][:].rearrange("p (w r j) -> p w r j", w=NWB, r=WS, j=WS)
                    evac_copy((bi + cc) % 2, out_ap, in_ap)
                # ---- q,k projection
                qkb = [p_qkb.tile([128, BAND], attn_dt, tag="qkb", name="qkb") for _ in range(6)]
                for half in range(2):
                    for oc in range(6):
                        ps = ps_proj.tile([128, HALF], f32, tag="psproj", name="psproj", padded_shape=[128, 512])
                        for cc in range(3):
                            nc.tensor.matmul(
                                ps[:], wqk[d][cc][:, oc * 128:(oc + 1) * 128],
                                xw[cc][:, half * HALF:(half + 1) * HALF],
                                start=(cc == 0), stop=(cc == 2))
                        evac_bias((oc + half) % 2,
                                  qkb[oc][:, half * HALF:(half + 1) * HALF],
                                  ps[:], bqk[d][:, oc:oc + 1])
                # ---- window-pair groups
                outT = p_outT.tile([128, 3 * BAND], f32r, tag="outT", name="outT")
                # ---- V projection for all 8 groups up front (DMA remap hides)
                vts = []
                for g in range(NG):
                    vt = p_vt.tile([128, 2 * 12 * 33], attn_dt, tag="vt", name="vt")
                    vp = ps_v.tile([128, C], f32, tag="psv", name="psv", padded_shape=[128, 512])
                    for cc in range(3):
                        nc.tensor.matmul(vp[0:98, :],
                                         xw[cc][:, 98 * g:98 * g + 98],
                                         wv[d][cc][:],
                                         start=(cc == 0), stop=(cc == 2))
                    iv = vp[0:98, :].rearrange("p (h e) -> p h e", h=12)
                    ov = vt[0:98, 0:396].rearrange("p (h e) -> p h e", e=33)[:, :, 0:32]
                    evac_copy(g % 2, ov, iv)
                    nc.vector.memset(
                        vt[:, 0:396].rearrange("p (h e) -> p h e", e=33)[:, :, 32:33],
                        1.0)
                    nc.sync.dma_start(vt[64:113, 396:792], vt[49:98, 0:396])
                    vts.append(vt)
                for g in range(NG):
                    vt = vts[g]
                    # S^T in one 4-bank psum tile; bank r = row group h%4.
                    # bias+mask preloaded (log domain); QK accumulates onto it.
                    if d == 0:
                        et, blk = e0, 0
                    elif g < NG - 1:
                        et, blk = e1, (0 if bi < NBAND - 1 else 1)
                    else:
                        et, blk = e1, (2 if bi < NBAND - 1 else 3)
                    ebase = blk * 12 * N
                    sp = ps_s.tile([128, 2048], f32, tag="pss", name="pss")
                    ea = et[:].rearrange("p f -> p f")
                    pre_in = bass.AP(ea.tensor, ea.offset + ebase,
                                     [ea.ap[0], [N, 4], [4 * N, 3], [1, N]])
                    pre_out = bass.AP(sp[:].tensor, sp[:].offset,
                                      [sp[:].ap[0], [512, 4], [N, 3], [1, N]])
                    if g % 2 == 0:
                        nc.scalar.activation(pre_out, pre_in, AF.Identity, bias=0.0)
                    else:
                        nc.vector.tensor_copy(pre_out, pre_in)
                    for s in range(2):
                        w = 2 * g + s
                        for h in range(NH):
                            po = 32 * (h % 4)
                            kT = qkb[3 + h // 4][po:po + 32, N * w:N * w + N]
                            qT = qkb[h // 4][po:po + 32, N * w:N * w + N]
                            out = sp[64 * s:64 * s + 49,
                                     512 * (h % 4) + N * (h // 4):
                                     512 * (h % 4) + N * (h // 4) + N]
                            nc.tensor.matmul(out, kT, qT, start=False, stop=True,
                                             tile_position=(po, 64 * s))
                    # single exp: psum -> bf16 pt at 52-pitch
                    pt = p_pt.tile([128, 12 * 52], attn_dt, tag="pt", name="pt")
                    if simsafe:
                        nc.vector.memset(pt[32:64, :], 0.0)
                        nc.vector.memset(pt[96:128, :], 0.0)
                    exp_in = bass.AP(sp[:].tensor, sp[:].offset,
                                     [sp[:].ap[0], [512, 4], [N, 3], [1, N]])
                    po_ = pt[:].rearrange("p f -> p f")
                    exp_out = bass.AP(po_.tensor, po_.offset,
                                      [po_.ap[0], [52, 4], [4 * 52, 3], [1, N]])
                    nc.scalar.activation(exp_out, exp_in, AF.Exp)
                    ptm = pt
                    # AV (+ fused row-sums via ones column of vt)
                    av = ps_av.tile([128, 12 * 33], f32, tag="psav", name="psav", padded_shape=[128, 512])
                    if simsafe:
                        nc.vector.memset(av[32:64, :], 0.0)
                        nc.vector.memset(av[96:128, :], 0.0)
                    for s in range(2):
                        for h in range(NH):
                            nc.tensor.matmul(
                                av[64 * s:64 * s + 49, 33 * h:33 * h + 33],
                                ptm[64 * s:64 * s + 49, 52 * h:52 * h + N],
                                vt[64 * s:64 * s + 49,
                                   396 * s + 33 * h:396 * s + 33 * h + 33],
                                start=True, stop=True,
                                tile_position=(64 * s, 64 * s))
                    # normalize
                    rt = p_rt.tile([128, 12], f32, tag="rt", name="rt")
                    nc.vector.reciprocal(
                        rt[:].rearrange("p (h e) -> p h e", e=1),
                        av[:].rearrange("p (h e) -> p h e", e=33)[:, :, 32:33])
                    on = p_on.tile([128, C], attn_dt, tag="onat", name="onat")
                    rap = rt[:]
                    rbc = bass.AP(rap.tensor, rap.offset, [rap.ap[0], [1, 12], [0, 32]])
                    nc.vector.tensor_tensor(
                        on[:].rearrange("p (h e) -> p h e", e=32),
                        av[:].rearrange("p (h e) -> p h e", e=33)[:, :, 0:32],
                        rbc, ALU.mult)
                    # transpose out [n, c] -> [c, n]; one psum bank per slot
                    for s in range(2):
                        tp = ps_v.tile([128, 3 * 52], attn_dt, tag="psv", name="psv",
                                       padded_shape=[128, 512])
                        for cc in range(3):
                            nc.tensor.transpose(
                                tp[:, 52 * cc:52 * cc + N],
                                on[64 * s:64 * s + 49, cc * 128:(cc + 1) * 128],
                                idn[64 * s:64 * s + 49, :])
                        oT = outT[:].rearrange("p (c t) -> p c t", c=3)
                        out_ap = bass.AP(oT.tensor, oT.offset + 98 * g + N * s,
                                         [oT.ap[0], [BAND, 3], [1, N]])
                        in_ap = bass.AP(tp[:].tensor, tp[:].offset,
                                        [tp[:].ap[0], [52, 3], [1, N]])
                        evac_copy(s % 2, out_ap, in_ap)
                # ---- output projection (+ window-major -> raster relayout)
                yr = [p_yr.tile([128, BAND], ydt, tag="yr", name="yr") for _ in range(3)]
                for half in range(2):
                    for oc in range(3):
                        ps = ps_proj.tile([128, HALF], f32, tag="psproj", name="psproj", padded_shape=[128, 512])
                        for cc in range(3):
                            nc.tensor.matmul(
                                ps[:], wp[d][cc][:, oc * 128:(oc + 1) * 128],
                                outT[:, cc * BAND + half * HALF:
                                     cc * BAND + (half + 1) * HALF],
                                start=(cc == 0), stop=(cc == 2))
                        in_ap = ps[:].rearrange("p (w r j) -> p w r j",
                                                w=NG, r=WS, j=WS)
                        out_ap = yr[oc][:].rearrange(
                            "p (r w j) -> p w r j", r=WS, w=NWB,
                            j=WS)[:, NG * half:NG * half + NG, :, :]
                        evac_bias((oc + half) % 2, out_ap, in_ap,
                                  bp[d][:, oc:oc + 1])
                # ---- store band
                for oc in range(3):
                    yr3 = yr[oc][:].rearrange("p (r c) -> p r c", r=WS)
                    dst3 = ydst[oc * 128:(oc + 1) * 128, :].rearrange(
                        "p (r c) -> p r c", r=H)
                    for (dr, sr, nr) in rows:
                        for (dc, sc, ncl) in cols:
                            nc.sync.dma_start(dst3[:, sr:sr + nr, sc:sc + ncl],
                                              yr3[:, dr:dr + nr, dc:dc + ncl])
            if d == 0 and ndepth > 1:
                tc.strict_bb_all_engine_barrier()

        for p in (ps_av, ps_s, ps_v, ps_proj, p_rt, p_on, p_vt,
                  p_pt, p_yr, p_outT, p_qkb, p_xw, p_xr, cpool):
            p.release()

    nc.compile()
    return nc


_NC = None


def _get_nc():
    global _NC
    if _NC is None:
        _NC = _build()
    return _NC


def _host_prep(qkv_w, qkv_b, proj_w, proj_b, rpb_table):
    scale = HD ** -0.5
    rpi = _rel_pos_index()
    masks = _masks()
    common = {}
    wqk = np.zeros((DEPTH, C, 2 * C), np.float32)
    wvv = np.zeros((DEPTH, C, C), np.float32)
    wpp = np.zeros((DEPTH, C, C), np.float32)
    bqk = np.zeros((DEPTH, 2 * C), np.float32)
    bpp = np.zeros((DEPTH, C), np.float32)
    for d in range(DEPTH):
        wq = qkv_w[d][:2 * C].T.copy()        # [C, 2C] (q then k)
        wq[:, :C] *= scale
        wqk[d] = wq
        wvv[d] = qkv_w[d][2 * C:].T
        wpp[d] = proj_w[d].T
        bq = qkv_b[d][:2 * C].copy()
        bq[:C] *= scale
        bqk[d] = bq
        bv = qkv_b[d][2 * C:]
        bpp[d] = proj_b[d] + proj_w[d] @ bv
    common["wqk"] = _round_f32r(wqk)
    common["wv"] = _round_f32r(wvv)
    common["wp"] = _round_f32r(wpp)
    common["bqk"] = bqk
    common["bp"] = bpp

    # E tiles: rows 0-48 -> m, rows 64-112 -> m-64; value exp(bias[h,n,m]+mask[n,m])
    def etile(d, type_a, type_b):
        bias = rpb_table[d][rpi]              # [N, N, NH]
        t = np.zeros((128, 12 * N), np.float32)
        for s, ty in ((0, type_a), (1, type_b)):
            bm = bias + (masks[ty][:, :, None] if ty is not None else 0.0)
            ev = bm.transpose(2, 1, 0)   # [NH, m, n] (log domain)
            blk = ev.transpose(1, 0, 2).reshape(N, 12 * N)  # row m, col h*N+n
            t[64 * s:64 * s + N, :] = blk
        return t

    common["e0"] = etile(0, None, None)
    e1 = np.zeros((128, 4 * 12 * N), np.float32)
    for b_, (ta, tb) in enumerate(((0, 0), (2, 2), (0, 1), (2, 3))):
        e1[:, b_ * 12 * N:(b_ + 1) * 12 * N] = etile(1, ta, tb)
    common["e1"] = e1

    idn = np.zeros((128, N), np.float32)
    idn[0:N, :] = np.eye(N, dtype=np.float32)
    idn[64:64 + N, :] = np.eye(N, dtype=np.float32)
    common["idn"] = idn.astype(ml_dtypes.bfloat16)
    return common


def kernel(x, qkv_w, qkv_b, proj_w, proj_b, rpb_table, H=None, W=None):
    x = np.asarray(x, np.float32)
    qkv_w = np.asarray(qkv_w, np.float32)
    qkv_b = np.asarray(qkv_b, np.float32)
    proj_w = np.asarray(proj_w, np.float32)
    proj_b = np.asarray(proj_b, np.float32)
    rpb_table = np.asarray(rpb_table, np.float32)

    nc = _get_nc()
    common = _host_prep(qkv_w, qkv_b, proj_w, proj_b, rpb_table)
    in_maps = []
    for b in range(B):
        m = dict(common)
        m["xin"] = _round_f32r(np.ascontiguousarray(x[b].T))
        in_maps.append(m)
    res = run_bass_kernel_spmd(nc, in_maps, core_ids=list(range(B)))
    out = np.stack([np.ascontiguousarray(res.results[b]["xout"].T)
                    for b in range(B)])
    return out.astype(np.float32)



# revision 6
# speedup vs baseline: 1.2987x; 1.2987x over previous
"""Swin BasicLayer (depth=2 windowed attention) Trainium2 kernel.

Sharding: data-parallel over batch B=8 across 8 NeuronCores; weights
replicated. Each core runs both depths (regular + shifted windows) over
its [12544, 384] image in [C, token] layout.
"""
import numpy as np
import ml_dtypes

import concourse.bass as bass
import concourse.tile as tile
from concourse import bacc, mybir
from concourse.bass_utils import run_bass_kernel_spmd

f32 = mybir.dt.float32
f32r = mybir.dt.float32r
bf16 = mybir.dt.bfloat16
AF = mybir.ActivationFunctionType
ALU = mybir.AluOpType

B, H, W, C = 8, 112, 112, 384
NH, HD, WS = 12, 32, 7
N = WS * WS          # 49
L = H * W            # 12544
NBAND = H // WS      # 16
BAND = WS * W        # 784
HALF = BAND // 2     # 392
NWB = W // WS        # 16 windows per band
NG = NWB // 2        # 8 window-pair groups
DEPTH = 2


def _round_f32r(x):
    v = np.ascontiguousarray(x, np.float32).view(np.uint32)
    v = ((v.astype(np.uint64) + 0x800) & 0xFFFFF000).astype(np.uint32)
    return v.view(np.float32)


def _rel_pos_index():
    coords = np.stack(np.meshgrid(np.arange(WS), np.arange(WS), indexing='ij')).reshape(2, -1)
    rel = (coords[:, :, None] - coords[:, None, :]).transpose(1, 2, 0).copy()
    rel[..., 0] += WS - 1
    rel[..., 1] += WS - 1
    rel[..., 0] *= 2 * WS - 1
    return rel.sum(-1)


def _masks():
    """The 4 distinct [N, N] shifted-window masks by type 2*(i==15)+(j==15)."""
    ws, shift = WS, WS // 2
    img = np.zeros((H, W), dtype=np.float32)
    slices = (slice(0, -ws), slice(-ws, -shift), slice(-shift, None))
    cnt = 0
    for hs in slices:
        for wsl in slices:
            img[hs, wsl] = cnt
            cnt += 1
    mw = img.reshape(H // ws, ws, W // ws, ws).transpose(0, 2, 1, 3).reshape(-1, ws * ws)
    diff = mw[:, None, :] - mw[:, :, None]
    mask = np.where(diff != 0, -100.0, 0.0).astype(np.float32)  # [nW, N, N]
    nwr = H // ws
    m = {}
    for ti, widx in ((0, 0), (1, nwr - 1), (2, (nwr - 1) * nwr), (3, nwr * nwr - 1)):
        m[ti] = mask[widx]
    return m


def _band_ranges(i, shift):
    r0 = (WS * i + shift) % H
    n0 = min(WS, H - r0)
    rows = [(0, r0, n0)]
    if n0 < WS:
        rows.append((n0, 0, WS - n0))
    if shift == 0:
        cols = [(0, 0, W)]
    else:
        cols = [(0, shift, W - shift), (W - shift, 0, shift)]
    return rows, cols


def _build(nbands=NBAND, ndepth=DEPTH, simsafe=False, attn_dt=bf16, proj_dt=bf16):
    nc = bacc.Bacc("TRN2", target_bir_lowering=False, debug=False, num_devices=8)

    xin = nc.dram_tensor("xin", [C, L], f32r, kind="ExternalInput")
    xmid = nc.dram_tensor("xmid", [C, L], f32r)
    xout = nc.dram_tensor("xout", [C, L], f32, kind="ExternalOutput")
    dwqk = nc.dram_tensor("wqk", [DEPTH, C, 2 * C], proj_dt, kind="ExternalInput")
    dwv = nc.dram_tensor("wv", [DEPTH, C, C], proj_dt, kind="ExternalInput")
    dwp = nc.dram_tensor("wp", [DEPTH, C, C], proj_dt, kind="ExternalInput")
    dbqk = nc.dram_tensor("bqk", [DEPTH, 2 * C], f32, kind="ExternalInput")
    dbp = nc.dram_tensor("bp", [DEPTH, C], f32, kind="ExternalInput")
    de0 = nc.dram_tensor("e0", [128, 12 * N], f32, kind="ExternalInput")
    de1 = nc.dram_tensor("e1", [128, 4 * 12 * N], f32, kind="ExternalInput")
    didn = nc.dram_tensor("idn", [128, N], attn_dt, kind="ExternalInput")

    def evac_copy(use_dve, out_ap, in_ap):
        if use_dve:
            nc.vector.tensor_copy(out_ap, in_ap)
        else:
            nc.scalar.activation(out_ap, in_ap, AF.Identity, bias=0.0)

    def evac_bias(use_dve, out_ap, in_ap, bias_ap):
        if use_dve:
            nc.vector.tensor_scalar_add(out_ap, in_ap, bias_ap)
        else:
            nc.scalar.activation(out_ap, in_ap, AF.Identity, bias=bias_ap)

    with tile.TileContext(nc) as tc:
        cpool = tc.alloc_tile_pool(name="const", bufs=1)
        p_xr = tc.alloc_tile_pool(name="xr", bufs=4)
        p_xw = tc.alloc_tile_pool(name="xw", bufs=6)
        p_qkb = tc.alloc_tile_pool(name="qkb", bufs=8)
        p_outT = tc.alloc_tile_pool(name="outT", bufs=6)
        p_yr = tc.alloc_tile_pool(name="yr", bufs=4)
        p_pt = tc.alloc_tile_pool(name="pt", bufs=3)
        p_vt = tc.alloc_tile_pool(name="vt", bufs=8)
        p_on = tc.alloc_tile_pool(name="onat", bufs=3)
        p_rt = tc.alloc_tile_pool(name="rt", bufs=3)
        ps_proj = tc.alloc_tile_pool(name="psproj", bufs=1, space="PSUM")
        ps_v = tc.alloc_tile_pool(name="psv", bufs=2, space="PSUM")
        ps_s = tc.alloc_tile_pool(name="pss", bufs=1, space="PSUM")
        ps_av = tc.alloc_tile_pool(name="psav", bufs=1, space="PSUM")

        # constants
        wqk = [[cpool.tile([128, 2 * C], proj_dt, tag=f"wqk{d}{cc}", name=f"wqk{d}{cc}") for cc in range(3)]
               for d in range(DEPTH)]
        wv = [[cpool.tile([128, C], proj_dt, tag=f"wv{d}{cc}", name=f"wv{d}{cc}") for cc in range(3)]
              for d in range(DEPTH)]
        wp = [[cpool.tile([128, C], proj_dt, tag=f"wp{d}{cc}", name=f"wp{d}{cc}") for cc in range(3)]
              for d in range(DEPTH)]
        bqk = [cpool.tile([128, 6], f32, tag=f"bqk{d}", name=f"bqk{d}") for d in range(DEPTH)]
        bp = [cpool.tile([128, 3], f32, tag=f"bp{d}", name=f"bp{d}") for d in range(DEPTH)]
        e0 = cpool.tile([128, 12 * N], f32, tag="e0", name="e0")
        e1 = cpool.tile([128, 4 * 12 * N], f32, tag="e1", name="e1")
        idn = cpool.tile([128, N], attn_dt, tag="idn", name="idn")
        for d in range(DEPTH):
            for cc in range(3):
                nc.sync.dma_start(wqk[d][cc][:], dwqk[d, cc * 128:(cc + 1) * 128, :])
                nc.sync.dma_start(wv[d][cc][:], dwv[d, cc * 128:(cc + 1) * 128, :])
                nc.sync.dma_start(wp[d][cc][:], dwp[d, cc * 128:(cc + 1) * 128, :])
            nc.sync.dma_start(bqk[d][:], dbqk[d].rearrange("(o p) -> p o", p=128))
            nc.sync.dma_start(bp[d][:], dbp[d].rearrange("(o p) -> p o", p=128))
        nc.sync.dma_start(e0[:], de0[:])
        nc.sync.dma_start(e1[:], de1[:])
        nc.sync.dma_start(idn[:], didn[:])

        for d in range(ndepth):
            shift = 0 if d == 0 else WS // 2
            xsrc = xin if d == 0 else xmid
            if d == ndepth - 1:
                ydst, ydt = xout, f32
            else:
                ydst, ydt = xmid, f32r
            for bi in range(nbands):
                rows, cols = _band_ranges(bi, shift)
                # ---- load band (raster) + window-major relayout
                xr = [p_xr.tile([128, BAND], f32r, tag="xr", name="xr") for _ in range(3)]
                xw = [p_xw.tile([128, BAND], proj_dt, tag="xw", name="xw") for _ in range(3)]
                for cc in range(3):
                    xr3 = xr[cc][:].rearrange("p (r c) -> p r c", r=WS)
                    src3 = xsrc[cc * 128:(cc + 1) * 128, :].rearrange(
                        "p (r c) -> p r c", r=H)
                    for (dr, sr, nr) in rows:
                        for (dc, sc, ncl) in cols:
                            nc.sync.dma_start(xr3[:, dr:dr + nr, dc:dc + ncl],
                                              src3[:, sr:sr + nr, sc:sc + ncl])
                    # raster (r, 7w+j) -> window-major (49w + 7r + j)
                    in_ap = xr[cc][:].rearrange("p (r w j) -> p w r j", r=WS, w=NWB, j=WS)
                    out_ap = xw[cc][:].rearrange("p (w r j) -> p w r j", w=NWB, r=WS, j=WS)
                    evac_copy((bi + cc) % 2, out_ap, in_ap)
                # ---- q,k projection
                qkb = [p_qkb.tile([128, BAND], attn_dt, tag="qkb", name="qkb") for _ in range(6)]
                for half in range(2):
                    for oc in range(6):
                        ps = ps_proj.tile([128, HALF], f32, tag="psproj", name="psproj", padded_shape=[128, 512])
                        for cc in range(3):
                            nc.tensor.matmul(
                                ps[:], wqk[d][cc][:, oc * 128:(oc + 1) * 128],
                                xw[cc][:, half * HALF:(half + 1) * HALF],
                                start=(cc == 0), stop=(cc == 2))
                        evac_bias((oc + half) % 2,
                                  qkb[oc][:, half * HALF:(half + 1) * HALF],
                                  ps[:], bqk[d][:, oc:oc + 1])
                # ---- window-pair groups
                outT = p_outT.tile([128, 3 * BAND], proj_dt, tag="outT", name="outT")
                # ---- V projection for all 8 groups up front (DMA remap hides)
                vts = []
                for g in range(NG):
                    vt = p_vt.tile([128, 2 * 12 * 33], attn_dt, tag="vt", name="vt")
                    vp = ps_v.tile([128, C], f32, tag="psv", name="psv", padded_shape=[128, 512])
                    for cc in range(3):
                        nc.tensor.matmul(vp[0:98, :],
                                         xw[cc][:, 98 * g:98 * g + 98],
                                         wv[d][cc][:],
                                         start=(cc == 0), stop=(cc == 2))
                    iv = vp[0:98, :].rearrange("p (h e) -> p h e", h=12)
                    ov = vt[0:98, 0:396].rearrange("p (h e) -> p h e", e=33)[:, :, 0:32]
                    evac_copy(g % 2, ov, iv)
                    nc.vector.memset(
                        vt[:, 0:396].rearrange("p (h e) -> p h e", e=33)[:, :, 32:33],
                        1.0)
                    nc.sync.dma_start(vt[64:113, 396:792], vt[49:98, 0:396])
                    vts.append(vt)
                for g in range(NG):
                    vt = vts[g]
                    # S^T in one 4-bank psum tile; bank r = row group h%4.
                    # bias+mask preloaded (log domain); QK accumulates onto it.
                    if d == 0:
                        et, blk = e0, 0
                    elif g < NG - 1:
                        et, blk = e1, (0 if bi < NBAND - 1 else 1)
                    else:
                        et, blk = e1, (2 if bi < NBAND - 1 else 3)
                    ebase = blk * 12 * N
                    sp = ps_s.tile([128, 2048], f32, tag="pss", name="pss")
                    ea = et[:].rearrange("p f -> p f")
                    pre_in = bass.AP(ea.tensor, ea.offset + ebase,
                                     [ea.ap[0], [N, 4], [4 * N, 3], [1, N]])
                    pre_out = bass.AP(sp[:].tensor, sp[:].offset,
                                      [sp[:].ap[0], [512, 4], [N, 3], [1, N]])
                    if g % 2 == 0:
                        nc.scalar.activation(pre_out, pre_in, AF.Identity, bias=0.0)
                    else:
                        nc.vector.tensor_copy(pre_out, pre_in)
                    for s in range(2):
                        w = 2 * g + s
                        for h in range(NH):
                            po = 32 * (h % 4)
                            kT = qkb[3 + h // 4][po:po + 32, N * w:N * w + N]
                            qT = qkb[h // 4][po:po + 32, N * w:N * w + N]
                            out = sp[64 * s:64 * s + 49,
                                     512 * (h % 4) + N * (h // 4):
                                     512 * (h % 4) + N * (h // 4) + N]
                            nc.tensor.matmul(out, kT, qT, start=False, stop=True,
                                             tile_position=(po, 64 * s))
                    # single exp: psum -> bf16 pt at 52-pitch
                    pt = p_pt.tile([128, 12 * 52], attn_dt, tag="pt", name="pt")
                    if simsafe:
                        nc.vector.memset(pt[32:64, :], 0.0)
                        nc.vector.memset(pt[96:128, :], 0.0)
                    exp_in = bass.AP(sp[:].tensor, sp[:].offset,
                                     [sp[:].ap[0], [512, 4], [N, 3], [1, N]])
                    po_ = pt[:].rearrange("p f -> p f")
                    exp_out = bass.AP(po_.tensor, po_.offset,
                                      [po_.ap[0], [52, 4], [4 * 52, 3], [1, N]])
                    nc.scalar.activation(exp_out, exp_in, AF.Exp)
                    ptm = pt
                    # AV (+ fused row-sums via ones column of vt)
                    av = ps_av.tile([128, 12 * 33], f32, tag="psav", name="psav", padded_shape=[128, 512])
                    if simsafe:
                        nc.vector.memset(av[32:64, :], 0.0)
                        nc.vector.memset(av[96:128, :], 0.0)
                    for s in range(2):
                        for h in range(NH):
                            nc.tensor.matmul(
                                av[64 * s:64 * s + 49, 33 * h:33 * h + 33],
                                ptm[64 * s:64 * s + 49, 52 * h:52 * h + N],
                                vt[64 * s:64 * s + 49,
                                   396 * s + 33 * h:396 * s + 33 * h + 33],
                                start=True, stop=True,
                                tile_position=(64 * s, 64 * s))
                    # normalize
                    rt = p_rt.tile([128, 12], f32, tag="rt", name="rt")
                    nc.vector.reciprocal(
                        rt[:].rearrange("p (h e) -> p h e", e=1),
                        av[:].rearrange("p (h e) -> p h e", e=33)[:, :, 32:33])
                    on = p_on.tile([128, C], attn_dt, tag="onat", name="onat")
                    rap = rt[:]
                    rbc = bass.AP(rap.tensor, rap.offset, [rap.ap[0], [1, 12], [0, 32]])
                    nc.vector.tensor_tensor(
                        on[:].rearrange("p (h e) -> p h e", e=32),
                        av[:].rearrange("p (h e) -> p h e", e=33)[:, :, 0:32],
                        rbc, ALU.mult)
                    # transpose out [n, c] -> [c, n]; one psum bank per slot
                    for s in range(2):
                        tp = ps_v.tile([128, 3 * 52], attn_dt, tag="psv", name="psv",
                                       padded_shape=[128, 512])
                        for cc in range(3):
                            nc.tensor.transpose(
                                tp[:, 52 * cc:52 * cc + N],
                                on[64 * s:64 * s + 49, cc * 128:(cc + 1) * 128],
                                idn[64 * s:64 * s + 49, :])
                        oT = outT[:].rearrange("p (c t) -> p c t", c=3)
                        out_ap = bass.AP(oT.tensor, oT.offset + 98 * g + N * s,
                                         [oT.ap[0], [BAND, 3], [1, N]])
                        in_ap = bass.AP(tp[:].tensor, tp[:].offset,
                                        [tp[:].ap[0], [52, 3], [1, N]])
                        evac_copy(s % 2, out_ap, in_ap)
                # ---- output projection (+ window-major -> raster relayout)
                yr = [p_yr.tile([128, BAND], ydt, tag="yr", name="yr") for _ in range(3)]
                for half in range(2):
                    for oc in range(3):
                        ps = ps_proj.tile([128, HALF], f32, tag="psproj", name="psproj", padded_shape=[128, 512])
                        for cc in range(3):
                            nc.tensor.matmul(
                                ps[:], wp[d][cc][:, oc * 128:(oc + 1) * 128],
                                outT[:, cc * BAND + half * HALF:
                                     cc * BAND + (half + 1) * HALF],
                                start=(cc == 0), stop=(cc == 2))
                        in_ap = ps[:].rearrange("p (w r j) -> p w r j",
                                                w=NG, r=WS, j=WS)
                        out_ap = yr[oc][:].rearrange(
                            "p (r w j) -> p w r j", r=WS, w=NWB,
                            j=WS)[:, NG * half:NG * half + NG, :, :]
                        evac_bias((oc + half) % 2, out_ap, in_ap,
                                  bp[d][:, oc:oc + 1])
                # ---- store band
                for oc in range(3):
                    yr3 = yr[oc][:].rearrange("p (r c) -> p r c", r=WS)
                    dst3 = ydst[oc * 128:(oc + 1) * 128, :].rearrange(
                        "p (r c) -> p r c", r=H)
                    for (dr, sr, nr) in rows:
                        for (dc, sc, ncl) in cols:
                            nc.sync.dma_start(dst3[:, sr:sr + nr, sc:sc + ncl],
                                              yr3[:, dr:dr + nr, dc:dc + ncl])
            if d == 0 and ndepth > 1:
                tc.strict_bb_all_engine_barrier()

        for p in (ps_av, ps_s, ps_v, ps_proj, p_rt, p_on, p_vt,
                  p_pt, p_yr, p_outT, p_qkb, p_xw, p_xr, cpool):
            p.release()

    nc.compile()
    return nc


_NC = None


def _get_nc():
    global _NC
    if _NC is None:
        _NC = _build()
    return _NC


def _host_prep(qkv_w, qkv_b, proj_w, proj_b, rpb_table):
    scale = HD ** -0.5
    rpi = _rel_pos_index()
    masks = _masks()
    common = {}
    wqk = np.zeros((DEPTH, C, 2 * C), np.float32)
    wvv = np.zeros((DEPTH, C, C), np.float32)
    wpp = np.zeros((DEPTH, C, C), np.float32)
    bqk = np.zeros((DEPTH, 2 * C), np.float32)
    bpp = np.zeros((DEPTH, C), np.float32)
    for d in range(DEPTH):
        wq = qkv_w[d][:2 * C].T.copy()        # [C, 2C] (q then k)
        wq[:, :C] *= scale
        wqk[d] = wq
        wvv[d] = qkv_w[d][2 * C:].T
        wpp[d] = proj_w[d].T
        bq = qkv_b[d][:2 * C].copy()
        bq[:C] *= scale
        bqk[d] = bq
        bv = qkv_b[d][2 * C:]
        bpp[d] = proj_b[d] + proj_w[d] @ bv
    common["wqk"] = wqk.astype(ml_dtypes.bfloat16)
    common["wv"] = wvv.astype(ml_dtypes.bfloat16)
    common["wp"] = wpp.astype(ml_dtypes.bfloat16)
    common["bqk"] = bqk
    common["bp"] = bpp

    # E tiles: rows 0-48 -> m, rows 64-112 -> m-64; value exp(bias[h,n,m]+mask[n,m])
    def etile(d, type_a, type_b):
        bias = rpb_table[d][rpi]              # [N, N, NH]
        t = np.zeros((128, 12 * N), np.float32)
        for s, ty in ((0, type_a), (1, type_b)):
            bm = bias + (masks[ty][:, :, None] if ty is not None else 0.0)
            ev = bm.transpose(2, 1, 0)   # [NH, m, n] (log domain)
            blk = ev.transpose(1, 0, 2).reshape(N, 12 * N)  # row m, col h*N+n
            t[64 * s:64 * s + N, :] = blk
        return t

    common["e0"] = etile(0, None, None)
    e1 = np.zeros((128, 4 * 12 * N), np.float32)
    for b_, (ta, tb) in enumerate(((0, 0), (2, 2), (0, 1), (2, 3))):
        e1[:, b_ * 12 * N:(b_ + 1) * 12 * N] = etile(1, ta, tb)
    common["e1"] = e1

    idn = np.zeros((128, N), np.float32)
    idn[0:N, :] = np.eye(N, dtype=np.float32)
    idn[64:64 + N, :] = np.eye(N, dtype=np.float32)
    common["idn"] = idn.astype(ml_dtypes.bfloat16)
    return common


def kernel(x, qkv_w, qkv_b, proj_w, proj_b, rpb_table, H=None, W=None):
    x = np.asarray(x, np.float32)
    qkv_w = np.asarray(qkv_w, np.float32)
    qkv_b = np.asarray(qkv_b, np.float32)
    proj_w = np.asarray(proj_w, np.float32)
    proj_b = np.asarray(proj_b, np.float32)
    rpb_table = np.asarray(rpb_table, np.float32)

    nc = _get_nc()
    common = _host_prep(qkv_w, qkv_b, proj_w, proj_b, rpb_table)
    in_maps = []
    for b in range(B):
        m = dict(common)
        m["xin"] = _round_f32r(np.ascontiguousarray(x[b].T))
        in_maps.append(m)
    res = run_bass_kernel_spmd(nc, in_maps, core_ids=list(range(B)))
    out = np.stack([np.ascontiguousarray(res.results[b]["xout"].T)
                    for b in range(B)])
    return out.astype(np.float32)

